# revision 7
# baseline (speedup 1.0000x reference)
"""Trainium2 Bass kernel for a 12-head attention block (B=2, N=2048, C=768).

Sharding: the 24 (batch, head) pairs are split across 8 NeuronCores —
4 cores per batch element, 3 heads per core (data + head/tensor parallel).
Each core computes qkv projections for its heads, the full attention for
its heads (the N x N score matrix is private to a core), and a *partial*
output projection over its heads' channels.  The host sums the 4 partial
projections per batch element (the tensor-parallel all-reduce) and adds
the bias.

Device algorithm (all matmuls in float32r = fp32 bits read at ~FP22 by
the PE, full-rate; accumulation fp32):

  xT [768, 2048] (x transposed on host)
  B:  qk^T  = W_qk^T.T @ xT  -> per-head tile [q^T(64 rows); k^T(64)] x 2048
      (attention scale 1/8 and b_q, b_k folded into W/bias on host)
  B2: v     = xT.T @ W_v^T   -> [2048, 3*65] with a column of ones per head
  C:  S^T[key, q] = k^T.T @ q^T        (per 128-key tile, 512-q chunk)
      P^T = exp(S^T)                   (ScalarE, no max subtraction:
                                        logits are in [-3, 3] by construction)
      ctx_u^T[d|den, q] += [v | 1].T @ P^T   (fused denominator row)
  D:  ctx^T = ctx_u^T[0:64] * (1/den) (reciprocal + partition-broadcast)
  E:  y[n, :] += ctx^T.T @ W_p^T      (partial projection, summed on host)
"""

import numpy as np

import concourse.bacc as bacc
import concourse.bass as bass
import concourse.tile as tile
import concourse.mybir as mybir
from concourse.bass_utils import run_bass_kernel_spmd

# Problem shape (hardcoded; harness contract)
B, N, C = 2, 2048, 768
H, HD = 12, 64
NCORES = 8
CORES_PER_B = NCORES // B      # 4
HPC = H // CORES_PER_B         # 3 heads per core
P = 128
NT = N // P                    # 16 key/n tiles
KT = C // P                    # 6 c_in tiles
CH = 512                       # q chunk (max fp32 moving free dim)
QCH = N // CH                  # 4 chunks
VW = 256                       # padded v width (>=256 keeps f32r at full rate)

f32 = mybir.dt.float32
f32r = mybir.dt.float32r
EXP = mybir.ActivationFunctionType.Exp

STAGE2 = True        # pack S^T matmul pairs into both halves of the PE array
NORM_GPSIMD = True   # broadcast 1/den via GpSimd; else PE ones-matmul


def _r(ap):
    return ap


def _emit(tc, nc, xT, w_qk, w_v, b_qk, w_p, vones, onesrow, y, dbg=None):
    from contextlib import ExitStack

    with ExitStack() as ctx:
        consts = ctx.enter_context(tc.tile_pool(name="consts", bufs=1))
        qk_pool = ctx.enter_context(tc.tile_pool(name="qk", bufs=HPC))
        qk2_pool = ctx.enter_context(tc.tile_pool(name="qk2", bufs=HPC))
        v_pool = ctx.enter_context(tc.tile_pool(name="v", bufs=NT))
        ctx_pool = ctx.enter_context(tc.tile_pool(name="ctxp", bufs=HPC))
        y_pool = ctx.enter_context(tc.tile_pool(name="y", bufs=3))
        ps_a = ctx.enter_context(tc.tile_pool(name="ps_a", bufs=2, space="PSUM"))
        ps_s = ctx.enter_context(tc.tile_pool(name="ps_s", bufs=3, space="PSUM"))
        ps_c = ctx.enter_context(tc.tile_pool(name="ps_c", bufs=2, space="PSUM"))

        # ---- constants
        wqk_sb = consts.tile([P, KT, 2 * HD * HPC], f32r)
        nc.sync.dma_start(wqk_sb[:], w_qk.rearrange("(t p) m -> p t m", p=P))
        wv_sb = consts.tile([P, KT, VW], f32r)
        nc.sync.dma_start(wv_sb[:], w_v.rearrange("(t p) m -> p t m", p=P))
        bqk_sb = consts.tile([P, HPC], f32)
        nc.sync.dma_start(bqk_sb[:], b_qk.rearrange("t p -> p t"))
        wp_sb = consts.tile([HD, HPC, C], f32r)
        nc.sync.dma_start(wp_sb[:], w_p.rearrange("(h p) m -> p h m", p=HD))
        vones_sb = consts.tile([P, VW], f32)
        nc.sync.dma_start(vones_sb[:], vones[:])
        onesrow_sb = consts.tile([HD + 1, HD], f32r)
        nc.sync.dma_start(onesrow_sb[:], onesrow[:])

        # persistent activations
        qk_sb = [qk_pool.tile([P, N], f32r, tag="qk", name=f"qk{_}") for _ in range(HPC)]
        qk2_sb = [qk2_pool.tile([P, N], f32r, tag="qk2", name=f"qk2_{_}") for _ in range(HPC)]
        v_sb = [v_pool.tile([P, VW], f32r, tag="v", name=f"v{_}") for _ in range(NT)]
        ctx_sb = [ctx_pool.tile([HD, N], f32r, tag="ctx", name=f"ctx{_}") for _ in range(HPC)]

        with tc.tile_pool(name="x", bufs=KT) as x_pool:
            x_sb = [x_pool.tile([P, N], f32r, tag="x", name=f"x{_}") for _ in range(KT)]
            for kt in range(KT):
                nc.sync.dma_start(x_sb[kt][:], xT[kt * P : (kt + 1) * P, :])

            # ---- B: qk^T per head: [q^T(64); k^T(64)] x N
            for t in range(HPC):
                for c in range(QCH):
                    ps = ps_a.tile([P, CH], f32, tag="ps_a")
                    for kt in range(KT):
                        nc.tensor.matmul(
                            ps[:],
                            _r(wqk_sb[:, kt, t * P : (t + 1) * P]),
                            _r(x_sb[kt][:, c * CH : (c + 1) * CH]),
                            start=(kt == 0),
                            stop=(kt == KT - 1),
                        )
                    nc.vector.tensor_scalar_add(
                        qk_sb[t][:, c * CH : (c + 1) * CH], ps[:], bqk_sb[:, t : t + 1]
                    )
                # swapped copy: k^T on partitions 0:64, q^T on 64:128
                nc.sync.dma_start(qk2_sb[t][0:HD, :], qk_sb[t][HD:P, :])
                nc.sync.dma_start(qk2_sb[t][HD:P, :], qk_sb[t][0:HD, :])
                if dbg is not None and t == 0:
                    nc.sync.dma_start(dbg["qk0"][:], qk_sb[0][:])
                    nc.sync.dma_start(dbg["qk20"][:], qk2_sb[0][:])

            # ---- B2: v natural layout [key, 3*65] (+ ones columns)
            for nt in range(NT):
                ps = ps_a.tile([P, CH], f32, tag="ps_a")
                for kt in range(KT):
                    nc.tensor.matmul(
                        ps[:, 0:VW],
                        _r(x_sb[kt][:, nt * P : (nt + 1) * P]),
                        _r(wv_sb[:, kt, :]),
                        start=(kt == 0),
                        stop=(kt == KT - 1),
                    )
                nc.vector.tensor_add(v_sb[nt][:], ps[:, 0:VW], vones_sb[:])
                if dbg is not None and nt == 0:
                    nc.sync.dma_start(dbg["v0"][:], v_sb[0][:])

        # x freed here
        recip_pool = ctx.enter_context(tc.tile_pool(name="recip", bufs=2))
        bc_pool = ctx.enter_context(tc.tile_pool(name="bc", bufs=2))
        p_pool = ctx.enter_context(tc.tile_pool(name="p", bufs=6))
        if dbg is not None:
            dbg["cps0_sb"] = bc_pool.tile([65, CH], f32, tag="cpsdump", bufs=1, name="cpsdump")

        # ---- C/D: attention per chunk; E: projection per chunk
        for c in range(QCH):
            for h in range(HPC):
                cps = ps_c.tile([65, CH], f32, tag="ps_c")
                for kt in range(NT):
                    sps = ps_s.tile([P, CH], f32, tag="ps_s")
                    if STAGE2 and (kt % 2 == 1):
                        # second half of the PE array (partitions 64:128)
                        nc.tensor.matmul(
                            sps[:],
                            _r(qk_sb[h][HD:P, kt * P : (kt + 1) * P]),
                            _r(qk2_sb[h][HD:P, c * CH : (c + 1) * CH]),
                        )
                    else:
                        nc.tensor.matmul(
                            sps[:],
                            _r(qk2_sb[h][0:HD, kt * P : (kt + 1) * P]),
                            _r(qk_sb[h][0:HD, c * CH : (c + 1) * CH]),
                        )
                    pt = p_pool.tile([P, CH], f32r, tag="p")
                    nc.scalar.activation(pt[:], sps[:], EXP)
                    if dbg is not None and c == 0 and h == 0 and kt < 2:
                        nc.sync.dma_start(dbg[f"p{kt}"][:], pt[:])
                    nc.tensor.matmul(
                        cps[:],
                        _r(v_sb[kt][:, h * 65 : (h + 1) * 65]),
                        _r(pt[:]),
                        start=(kt == 0),
                        stop=(kt == NT - 1),
                    )
                # D: normalize by the fused denominator row
                rc = recip_pool.tile([65, CH], f32r, tag="recip")
                with nc.allow_low_precision(reason="fp32r rounding of 1/denominator"):
                    nc.vector.reciprocal(rc[64:65, :], cps[64:65, :])
                bps = ps_s.tile([P, CH], f32, tag="ps_s")
                nc.tensor.matmul(
                    bps[0:HD, :], onesrow_sb[HD : HD + 1, :], rc[64:65, :],
                    start=True, stop=True,
                )
                bc = bc_pool.tile([HD, CH], f32, tag="bc")
                nc.vector.tensor_copy(bc[:], bps[0:HD, :])
                if dbg is not None and c == 0 and h == 0:
                    nc.vector.tensor_copy(dbg["cps0_sb"][:], cps[:])
                    nc.sync.dma_start(dbg["cps0"][:], dbg["cps0_sb"][:])
                    nc.sync.dma_start(dbg["bc0"][:], bc[:])
                nc.vector.tensor_mul(
                    ctx_sb[h][:, c * CH : (c + 1) * CH], cps[0:HD, :], bc[:]
                )

            if dbg is not None and c == QCH - 1:
                for hh in range(HPC):
                    nc.sync.dma_start(dbg[f"ctx{hh}"][:], ctx_sb[hh][:])
            # E: partial projection for this chunk's 4 n-tiles
            for i in range(CH // P):
                nt = c * (CH // P) + i
                psA = ps_a.tile([P, CH], f32, tag="ps_a")
                psB = ps_a.tile([P, CH], f32, tag="ps_a")
                for h in range(HPC):
                    nc.tensor.matmul(
                        psA[:],
                        _r(ctx_sb[h][:, nt * P : (nt + 1) * P]),
                        _r(wp_sb[:, h, 0:CH]),
                        start=(h == 0),
                        stop=(h == HPC - 1),
                    )
                for h in range(HPC):
                    nc.tensor.matmul(
                        psB[:, 0 : C - CH],
                        _r(ctx_sb[h][:, nt * P : (nt + 1) * P]),
                        _r(wp_sb[:, h, CH:C]),
                        start=(h == 0),
                        stop=(h == HPC - 1),
                    )
                ysb = y_pool.tile([P, C], f32, tag="y")
                nc.vector.tensor_copy(ysb[:, 0:CH], psA[:])
                nc.vector.tensor_copy(ysb[:, CH:C], psB[:, 0 : C - CH])
                nc.sync.dma_start(y[nt * P : (nt + 1) * P, :], ysb[:])


def build_program(debug=False):
    nc = bacc.Bacc("TRN2", target_bir_lowering=False, debug=False)
    xT = nc.dram_tensor("xT", [C, N], f32r, kind="ExternalInput").ap()
    w_qk = nc.dram_tensor("w_qk", [C, 2 * HD * HPC], f32r, kind="ExternalInput").ap()
    w_v = nc.dram_tensor("w_v", [C, VW], f32r, kind="ExternalInput").ap()
    b_qk = nc.dram_tensor("b_qk", [HPC, P], f32, kind="ExternalInput").ap()
    w_p = nc.dram_tensor("w_p", [HPC * HD, C], f32r, kind="ExternalInput").ap()
    vones = nc.dram_tensor("vones", [P, VW], f32, kind="ExternalInput").ap()
    onesrow = nc.dram_tensor("onesrow", [HD + 1, HD], f32r, kind="ExternalInput").ap()
    y = nc.dram_tensor("y", [N, C], f32, kind="ExternalOutput").ap()
    dbg = None
    if debug:
        dbg = {
            "qk0": nc.dram_tensor("dbg_qk0", [P, N], f32r, kind="ExternalOutput").ap(),
            "qk20": nc.dram_tensor("dbg_qk20", [P, N], f32r, kind="ExternalOutput").ap(),
            "v0": nc.dram_tensor("dbg_v0", [P, VW], f32r, kind="ExternalOutput").ap(),
            "p0": nc.dram_tensor("dbg_p0", [P, CH], f32r, kind="ExternalOutput").ap(),
            "p1": nc.dram_tensor("dbg_p1", [P, CH], f32r, kind="ExternalOutput").ap(),
            "cps0": nc.dram_tensor("dbg_cps0", [65, CH], f32, kind="ExternalOutput").ap(),
            "bc0": nc.dram_tensor("dbg_bc0", [HD, CH], f32, kind="ExternalOutput").ap(),
            "ctx0": nc.dram_tensor("dbg_ctx0", [HD, N], f32r, kind="ExternalOutput").ap(),
            "ctx1": nc.dram_tensor("dbg_ctx1", [HD, N], f32r, kind="ExternalOutput").ap(),
            "ctx2": nc.dram_tensor("dbg_ctx2", [HD, N], f32r, kind="ExternalOutput").ap(),
        }
    with tile.TileContext(nc) as tc:
        _emit(tc, nc, xT, w_qk, w_v, b_qk, w_p, vones, onesrow, y, dbg=dbg)
    nc.compile()
    return nc


_CACHE = {}


def _get_program():
    if "nc" not in _CACHE:
        _CACHE["nc"] = build_program()
    return _CACHE["nc"]


def make_in_maps(x, W_qkv, b_qkv, W_proj):
    """Per-core input dicts implementing the (batch, head-group) sharding."""
    x = np.ascontiguousarray(np.asarray(x, np.float32))
    W_qkv = np.asarray(W_qkv, np.float32)
    b_qkv = np.asarray(b_qkv, np.float32)
    W_proj = np.asarray(W_proj, np.float32)
    scale = float(HD) ** -0.5

    Wq = W_qkv[0:C].reshape(H, HD, C)
    Wk = W_qkv[C : 2 * C].reshape(H, HD, C)
    Wv = W_qkv[2 * C : 3 * C].reshape(H, HD, C)
    bq = b_qkv[0:C].reshape(H, HD)
    bk = b_qkv[C : 2 * C].reshape(H, HD)

    vones_mask = np.zeros((P, VW), np.float32)
    for i in range(HPC):
        vones_mask[:, i * 65 + HD] = 1.0
    onesrow_arr = np.zeros((HD + 1, HD), np.float32)
    onesrow_arr[HD, :] = 1.0

    in_maps = []
    for core in range(NCORES):
        b = core // CORES_PER_B
        hg = core % CORES_PER_B
        heads = list(range(hg * HPC, (hg + 1) * HPC))

        xT = np.ascontiguousarray(x[b].T)                       # [C, N]
        w_qk = np.empty((C, 2 * HD * HPC), np.float32)
        b_qk_arr = np.empty((HPC, P), np.float32)
        w_v = np.zeros((C, VW), np.float32)
        w_p = np.empty((HPC * HD, C), np.float32)
        for i, h in enumerate(heads):
            w_qk[:, i * P : i * P + HD] = Wq[h].T * scale
            w_qk[:, i * P + HD : (i + 1) * P] = Wk[h].T
            b_qk_arr[i, 0:HD] = bq[h] * scale
            b_qk_arr[i, HD:P] = bk[h]
            w_v[:, i * 65 : i * 65 + HD] = Wv[h].T
            w_p[i * HD : (i + 1) * HD, :] = W_proj[:, h * HD : (h + 1) * HD].T
        in_maps.append(
            {"xT": xT, "w_qk": w_qk, "w_v": w_v, "b_qk": b_qk_arr, "w_p": w_p,
             "vones": vones_mask, "onesrow": onesrow_arr}
        )
    return in_maps


def gather_output(results, b_qkv, W_proj, b_proj):
    """Sum the per-core partial projections (TP all-reduce) + effective bias."""
    out = np.zeros((B, N, C), np.float32)
    for core in range(NCORES):
        out[core // CORES_PER_B] += results[core]["y"]
    b_v = np.asarray(b_qkv, np.float32)[2 * C : 3 * C]
    b_eff = np.asarray(b_proj, np.float32) + np.asarray(W_proj, np.float32) @ b_v
    out += b_eff
    return out


def kernel(x=None, xpos=None, W_qkv=None, b_qkv=None, W_proj=None, b_proj=None, **kw):
    del xpos, kw  # rope disabled in this configuration; xpos unused
    nc = _get_program()
    in_maps = make_in_maps(x, W_qkv, b_qkv, W_proj)
    res = run_bass_kernel_spmd(nc, in_maps, core_ids=list(range(NCORES)))
    return gather_output(res.results, b_qkv, W_proj, b_proj)
